# revision 5
# baseline (speedup 1.0000x reference)
"""Trainium2 Bass kernel for AdaptiveNet MLP (fc1+sigmoid, grouped fc2+sigmoid, fc3).

Sharding: pure data-parallel over batch across 8 NeuronCores (no collectives).
Each core computes its 2048-row shard through all three layers.

fc1 (95% of FLOPs) runs in fp8-e4m3 with DoubleRow perf mode (two fp8 weights
per PE cell -> K=256 per matmul, halving the matmul count); the sigmoid damps
the quantization error so the final rel-err stays ~3e-3 (gate is 2e-2).

Layout trick: H1 is permuted s-major on the host (h1' = s*512 + g, where the
original h1 = g*8 + s).  fc1 then produces hT' tiles [128 h1' partitions x 512
rows]; the grouped fc2 contraction over s becomes 8 fused multiply-accumulate
ops on the vector engine with per-partition scalars (W2 columns), and fc3 is a
plain bf16 matmul over the 512 groups.  Biases are per-partition [128,1]
columns fused into ScalarE sigmoids / a VectorE add.

v4 overlap work (~2us over the first DR version): 12 throwaway DR matmuls on
a memset scratch tile warm the PE through the input-DMA head (HAM clock gate
flips to 8/8 before real work); the first column-block runs j-outer with
128KB j-pair DMAs so real matmuls start ~3us earlier; fc2 x2 sigmoids are
emitted one chunk late to dodge ACT-queue head-of-line blocking on their
vector STT; fc3 issues t-outer (in-order PE) with the last row-block's final
chunk pipelined at half width, and fc3(1) deferred last so its t0..t2
matmuls cover the final sigmoid chain.
"""

import sys

for _p in ("/opt/trn_rl_repo",):
    if _p not in sys.path:
        sys.path.append(_p)

import numpy as np
import ml_dtypes

BF16 = ml_dtypes.bfloat16
FP8 = ml_dtypes.float8_e4m3  # == mybir.dt.float8e4

D_IN, H1, H2, D_OUT = 1024, 4096, 512, 256
GS = H1 // H2  # 8
B = 16384
N_CORES = 8
B_SHARD = B // N_CORES  # 2048
NBLK = 512  # rows per block (one PSUM bank of fp32)
NB = B_SHARD // NBLK  # 4
KC = D_IN // 128  # 8 contraction subtiles for fc1
KP = KC // 2  # 4 DoubleRow pairs
CC = H1 // 128  # 32 h1' chunks
NT = H2 // 128  # 4 x2T tiles
ND = D_OUT // 128  # 2 output chunks

_compiled = {}


def _build_nc():
    from concourse import bacc, tile, mybir

    f32 = mybir.dt.float32
    bf16 = mybir.dt.bfloat16
    fp8 = mybir.dt.float8e4
    AF = mybir.ActivationFunctionType
    ALU = mybir.AluOpType
    DR = mybir.MatmulPerfMode.DoubleRow

    nc = bacc.Bacc("TRN2", target_bir_lowering=False, debug=False,
                   num_devices=N_CORES)

    xq = nc.dram_tensor("xq", [128, KC, B_SHARD], fp8, kind="ExternalInput")
    w1q = nc.dram_tensor("w1q", [128, KC, H1], fp8, kind="ExternalInput")
    # all [128, *] f32 constants packed on the free axis:
    # b1 (CC) | w2 (CC) | b2 (NT) | b3 (ND)
    cst = nc.dram_tensor("cst", [128, 2 * CC + NT + ND], f32,
                         kind="ExternalInput")
    w3q = nc.dram_tensor("w3q", [128, NT, D_OUT], bf16, kind="ExternalInput")
    out = nc.dram_tensor("out", [D_OUT, B_SHARD], bf16, kind="ExternalOutput")

    with tile.TileContext(nc) as tc:
        with (
            tc.tile_pool(name="wpool", bufs=1) as wpool,
            tc.tile_pool(name="xpool", bufs=1) as xpool,
            tc.tile_pool(name="hpool", bufs=8) as hpool,
            tc.tile_pool(name="accpool", bufs=1) as accpool,
            tc.tile_pool(name="x2pool", bufs=1) as x2pool,
            tc.tile_pool(name="opool", bufs=4) as opool,
            tc.tile_pool(name="psum_h", bufs=8, space="PSUM") as psum_h_pool,
        ):
            psum_o_pool = psum_h_pool
            w1_sb = wpool.tile([128, KC, H1], fp8, tag="w1")
            x_sb = [None] * NB
            for n in range(NB):
                x_sb[n] = xpool.tile([128, KC, NBLK], fp8,
                                     tag=f"x_{n}", name=f"xsb_{n}")
            # Three issuing queues (sync/scalar HWDGE, gpsimd SWDGE), FIFO
            # within each; ~2-3us per dma_start regardless of size, so use
            # few DMAs, landing in exactly the order the loop consumes:
            # W1 column-blocks (all subtile pairs of CBLK c-chunks in one
            # strided DMA) on sync; whole x tiles on scalar/gpsimd.
            CBLK = 4
            WBLK = 8

            cst_sb = wpool.tile([128, 2 * CC + NT + ND], f32, tag="cst")
            w3_sb = wpool.tile([128, NT, D_OUT], bf16, tag="w3q")

            # --- PE warmup: keep the PE busy through the input-DMA head so
            # the HAM clock gate flips to 8/8 before real matmuls arrive and
            # the cold 1.2GHz window is spent on throwaway work.  The scratch
            # tile is memset on the vector queue (idle at t=0, unlike the
            # DMA-issuing queues); 12 N=256 DR matmuls cover ~0.4-3us. ---
            wm = wpool.tile([128, 2, 256], fp8, tag="wm")
            nc.vector.memset(wm[:], 0)
            wm_ps = psum_h_pool.tile([128, 256], f32, tag="psum_h",
                                     name="wm_ps")
            for _wi in range(12):
                nc.tensor.matmul(wm_ps[:], lhsT=wm[:, :, 0:128], rhs=wm[:],
                                 start=True, stop=True,
                                 perf_mode=DR)

            def b1col(c):
                return cst_sb[:, c:c + 1]

            def w2col(c):
                return cst_sb[:, CC + c:CC + c + 1]

            def b2col(t):
                return cst_sb[:, 2 * CC + t:2 * CC + t + 1]

            def b3col(d):
                return cst_sb[:, 2 * CC + NT + d:2 * CC + NT + d + 1]

            # one DMA per W1 column-block covering all subtile pairs; the
            # first block split in half so the very first matmuls unblock
            # sooner
            def wblock(cb0, cb1, eng):
                c0, c1 = cb0 * 128, cb1 * 128
                eng.dma_start(w1_sb[:, :, c0:c1], w1q.ap()[:, :, c0:c1])

            def xdma(n, eng):
                eng.dma_start(x_sb[n][:],
                              xq.ap()[:, :, n * NBLK:(n + 1) * NBLK])

            # sync lane: W1 for the first column-block in j-pair slices
            # (128KB each) so the very first j-outer matmuls unblock at
            # ~2.5us instead of ~6us, then x_1/x_3, then the big W1 blocks
            # (consumed only from ~16us on)
            for j in range(KP):
                nc.sync.dma_start(
                    w1_sb[:, 2 * j:2 * j + 2, 0:CBLK * 128],
                    w1q.ap()[:, 2 * j:2 * j + 2, 0:CBLK * 128])
            xdma(1, nc.sync)
            xdma(3, nc.sync)
            for cb in range(CBLK, CC, WBLK):
                wblock(cb, min(cb + WBLK, CC), nc.sync)
            # scalar/gpsimd lanes: x_0 in j-pair slices (128KB) alternating
            # across both lanes so j=0's matmuls unblock first, then consts
            # and x_2/w3
            nc.scalar.dma_start(x_sb[0][:, 0:2, :], xq.ap()[:, 0:2, 0:NBLK])
            nc.gpsimd.dma_start(x_sb[0][:, 4:6, :], xq.ap()[:, 4:6, 0:NBLK])
            nc.scalar.dma_start(x_sb[0][:, 2:4, :], xq.ap()[:, 2:4, 0:NBLK])
            nc.gpsimd.dma_start(x_sb[0][:, 6:8, :], xq.ap()[:, 6:8, 0:NBLK])
            nc.scalar.dma_start(cst_sb[:], cst.ap()[:])
            xdma(2, nc.gpsimd)
            nc.gpsimd.dma_start(w3_sb[:], w3q.ap()[:])

            # fc2 accumulators, one per (row-block, x2 tile)
            acc = [[None] * NT for _ in range(NB)]

            x2_sb = [[None] * NT for _ in range(NB)]

            def fc2_step(c, n, ht):
                t_i = c % NT
                if c < NT:
                    acc[n][t_i] = accpool.tile([128, NBLK], bf16,
                                               tag=f"acc_{n}_{t_i}",
                                               name=f"acc_{n}_{t_i}")
                    nc.vector.tensor_scalar_mul(acc[n][t_i][:], ht[:],
                                                w2col(c))
                else:
                    nc.vector.scalar_tensor_tensor(
                        acc[n][t_i][:], ht[:], w2col(c),
                        acc[n][t_i][:], op0=ALU.mult, op1=ALU.add)
                if c >= CC - NT:
                    # chain for tile t_i is complete -> queue the fc2
                    # sigmoid, but emit it one chunk LATER: at the strict-
                    # FIFO ACT queue head it would stall on its vector STT
                    # (head-of-line blocking) while ready ht sigmoids sit
                    # behind it
                    pending_x2.append((n, t_i))

            pending_x2 = []

            def flush_x2(limit=None):
                k = 0
                while pending_x2 and (limit is None or k < limit):
                    n_, t_ = pending_x2.pop(0)
                    t = x2pool.tile([128, NBLK], bf16, tag=f"x2_{n_}_{t_}",
                                    name=f"x2sb_{n_}_{t_}")
                    nc.scalar.activation(t[:], acc[n_][t_][:], AF.Sigmoid,
                                         bias=b2col(t_))
                    x2_sb[n_][t_] = t
                    k += 1

            def fc1_block0():
                # j-outer over the first column-block of row-block 0: each
                # j-pair of x/W1 arrives as its own 128KB DMA, so matmuls
                # start as soon as the first pair lands (~2.5us) instead of
                # waiting for the full tiles
                phs = [psum_h_pool.tile([128, NBLK], f32, tag="psum_h",
                                        name=f"ph0_{c}")
                       for c in range(CBLK)]
                for j in range(KP):
                    for c in range(CBLK):
                        nc.tensor.matmul(
                            phs[c][:],
                            lhsT=w1_sb[:, 2 * j:2 * j + 2,
                                       128 * c:128 * (c + 1)],
                            rhs=x_sb[0][:, 2 * j:2 * j + 2, :],
                            start=(j == 0),
                            stop=(j == KP - 1),
                            perf_mode=DR,
                        )
                for c in range(CBLK):
                    ht = hpool.tile([128, NBLK], bf16, tag="ht",
                                    name=f"ht_0_{c}")
                    nc.scalar.activation(ht[:], phs[c][:], AF.Sigmoid,
                                         bias=b1col(c))
                    fc2_step(c, 0, ht)
                    flush_x2(limit=1)

            def fc1_block(cs, n, split_last=False):
                for c in cs:
                    split = split_last and c == cs[-1]
                    ph = psum_h_pool.tile([128, NBLK], f32, tag="psum_h",
                                          name=f"ph_{n}_{c}")
                    if not split:
                        for j in range(KP):
                            nc.tensor.matmul(
                                ph[:],
                                lhsT=w1_sb[:, 2 * j:2 * j + 2,
                                           128 * c:128 * (c + 1)],
                                rhs=x_sb[n][:, 2 * j:2 * j + 2, :],
                                start=(j == 0),
                                stop=(j == KP - 1),
                                perf_mode=DR,
                            )
                        ht = hpool.tile([128, NBLK], bf16, tag="ht",
                                        name=f"ht_{n}_{c}")
                        nc.scalar.activation(ht[:], ph[:], AF.Sigmoid,
                                             bias=b1col(c))
                        fc2_step(c, n, ht)
                        flush_x2(limit=1)
                        continue
                    # final chunk of the final row-block: half-width (256)
                    # pipeline so the serial ht->acc->x2 chain at the very
                    # end runs on half tiles, overlapped across halves
                    flush_x2()
                    t_i = c % NT
                    ht = hpool.tile([128, NBLK], bf16, tag="ht",
                                    name=f"ht_{n}_{c}")
                    t = x2pool.tile([128, NBLK], bf16, tag=f"x2_{n}_{t_i}",
                                    name=f"x2sb_{n}_{t_i}")
                    H = NBLK // 2
                    for h in range(2):
                        s = slice(h * H, (h + 1) * H)
                        for j in range(KP):
                            nc.tensor.matmul(
                                ph[:, s],
                                lhsT=w1_sb[:, 2 * j:2 * j + 2,
                                           128 * c:128 * (c + 1)],
                                rhs=x_sb[n][:, 2 * j:2 * j + 2, s],
                                start=(j == 0),
                                stop=(j == KP - 1),
                                perf_mode=DR,
                                skip_group_check=True,
                            )
                        nc.scalar.activation(ht[:, s], ph[:, s], AF.Sigmoid,
                                             bias=b1col(c))
                        nc.vector.scalar_tensor_tensor(
                            acc[n][t_i][:, s], ht[:, s], w2col(c),
                            acc[n][t_i][:, s], op0=ALU.mult, op1=ALU.add)
                        nc.scalar.activation(t[:, s], acc[n][t_i][:, s],
                                             AF.Sigmoid, bias=b2col(t_i))
                    x2_sb[n][t_i] = t

            def fc3_block(n, last=False):
                # t-outer issue order: the PE is in-order, so d-outer would
                # park the whole queue behind po[0]'s last accumulation step
                # (waiting on x2[n][NT-1]) while po[1]'s early steps could
                # already run.  t-outer drains all ready work first.
                po = [psum_o_pool.tile([128, NBLK], f32, tag="psum_h",
                                       name=f"po_{n}_{d}")
                      for d in range(ND)]
                for t_i in range(NT - 1):
                    for d in range(ND):
                        nc.tensor.matmul(
                            po[d][:],
                            lhsT=w3_sb[:, t_i, 128 * d:128 * (d + 1)],
                            rhs=x2_sb[n][t_i][:],
                            start=(t_i == 0),
                            stop=False,
                            skip_group_check=True,
                        )
                H = NBLK // 2
                for h in range(2):
                    s = slice(h * H, (h + 1) * H)
                    for d in range(ND):
                        nc.tensor.matmul(
                            po[d][:, s],
                            lhsT=w3_sb[:, NT - 1, 128 * d:128 * (d + 1)],
                            rhs=x2_sb[n][NT - 1][:, s],
                            start=False,
                            stop=(h == 1),
                            skip_group_check=True,
                        )
                for d in range(ND):
                    ot = opool.tile([128, NBLK], bf16, tag="ot",
                                    name=f"ot_{n}_{d}")
                    if last:
                        # half TS + half DMAs on distinct queues: the
                        # end-of-kernel exposure is one 64KB transfer
                        engs = ((nc.sync, nc.scalar) if d == 0
                                else (nc.gpsimd, nc.sync))
                        for h in range(2):
                            s = slice(h * H, (h + 1) * H)
                            nc.vector.tensor_scalar_add(ot[:, s], po[d][:, s],
                                                        b3col(d))
                            engs[h].dma_start(
                                out.ap()[128 * d:128 * (d + 1),
                                         n * NBLK + h * H:
                                         n * NBLK + (h + 1) * H],
                                ot[:, s])
                    else:
                        nc.vector.tensor_scalar_add(ot[:], po[d][:], b3col(d))
                        nc.sync.dma_start(
                            out.ap()[128 * d:128 * (d + 1),
                                     n * NBLK:(n + 1) * NBLK], ot[:])

            # --- fc1 + fc2: column-blocks of CBLK c-chunks, n-outer inside
            # so each x tile's DMA arrival unlocks a block of work; each
            # psum tile's 4 matmuls are consecutive (liveness ~1 bank).
            # In the last block, each row-block's fc3 is interleaved one
            # n-phase behind its fc1 so the x2 sigmoid chains are covered
            # by other matmul work. ---
            NORD = (0, 2, 1, 3)  # x DMA arrival order (2 lanes)
            # chain updates are commutative, so interleave the final c's
            # over the last two blocks: x2 sigmoids spread across 8 chunks
            # instead of bunching behind the last 4 (ACT would throttle PE
            # via PSUM slot release)
            c_seq = list(range(CC - 2 * CBLK)) + [24, 28, 25, 29, 26, 30,
                                                 27, 31]
            # last column-block runs n=1 LAST (order 0,2,3,1) and fc3(1) is
            # deferred to the very end: fc3(1)'s t0..t2 matmuls then cover
            # block 1's final ht->acc->x2 chain, so the PE never idles
            # waiting on the last sigmoid chain
            LAST_ORD = (0, 2, 3, 1)
            for b0 in range(0, CC, CBLK):
                last_b = b0 + CBLK == CC
                for n in (LAST_ORD if last_b else NORD):
                    if b0 == 0 and n == 0:
                        fc1_block0()
                    else:
                        fc1_block(c_seq[b0:b0 + CBLK], n,
                                  split_last=(last_b and n == LAST_ORD[-1]))
            flush_x2()
            for n in LAST_ORD:
                fc3_block(n, last=(n == LAST_ORD[-1]))

    nc.compile()
    return nc


def get_nc():
    if "nc" not in _compiled:
        _compiled["nc"] = _build_nc()
    return _compiled["nc"]


def make_in_maps(x, W1, b1, W2, b2, W3, b3):
    x = np.asarray(x, dtype=np.float32)
    W1 = np.asarray(W1, dtype=np.float32)
    b1 = np.asarray(b1, dtype=np.float32)
    W2 = np.asarray(W2, dtype=np.float32)
    b2 = np.asarray(b2, dtype=np.float32)
    W3 = np.asarray(W3, dtype=np.float32)
    b3 = np.asarray(b3, dtype=np.float32)

    # s-major permutation of H1: new index p = s*H2 + g  (old h1 = g*GS + s)
    p = np.arange(H1)
    perm = (p % H2) * GS + (p // H2)
    W1p = W1[perm, :]
    b1p = b1[perm]

    # fp8 fc1 operands in DoubleRow layout [128, KC, *]:
    # element (p, j, m) holds contraction index k = 128*j + p
    w1t = W1p.T.astype(FP8)  # [D_IN, H1]
    w1q_h = np.ascontiguousarray(
        w1t.reshape(KC, 128, H1).transpose(1, 0, 2))
    xt = x.T.astype(FP8)  # [D_IN, B]
    xq_h = np.ascontiguousarray(
        xt.reshape(KC, 128, B).transpose(1, 0, 2))

    b1c_h = b1p.reshape(CC, 128).T
    # chunk c: s = c//NT, tile t = c%NT, partition k <-> group 128*t + k
    w2c_h = np.empty((128, CC), dtype=np.float32)
    for c in range(CC):
        w2c_h[:, c] = W2[128 * (c % NT):128 * (c % NT) + 128, c // NT]
    b2c_h = b2.reshape(NT, 128).T
    b3c_h = b3.reshape(ND, 128).T
    cst_h = np.ascontiguousarray(
        np.concatenate([b1c_h, w2c_h, b2c_h, b3c_h], axis=1),
        dtype=np.float32)  # [128, 2*CC + NT + ND]
    w3t = W3.T.astype(BF16)  # [H2, D_OUT]
    w3q_h = np.ascontiguousarray(
        w3t.reshape(NT, 128, D_OUT).transpose(1, 0, 2))

    in_maps = []
    for i in range(N_CORES):
        in_maps.append({
            "xq": np.ascontiguousarray(
                xq_h[:, :, i * B_SHARD:(i + 1) * B_SHARD]),
            "w1q": w1q_h,
            "cst": cst_h,
            "w3q": w3q_h,
        })
    return in_maps


def kernel(x, W1, b1, W2, b2, W3, b3):
    import os
    from concourse.bass_utils import run_bass_kernel_spmd

    nc = get_nc()
    in_maps = make_in_maps(x, W1, b1, W2, b2, W3, b3)
    # force tracing off for this call: the agent image lacks the axon NTFF
    # hook module, so a stray BASS_TRACE=1 would crash the run
    prev = os.environ.get("BASS_NEVER_TRACE")
    os.environ["BASS_NEVER_TRACE"] = "1"
    try:
        res = run_bass_kernel_spmd(nc, in_maps, core_ids=list(range(N_CORES)))
    finally:
        if prev is None:
            os.environ.pop("BASS_NEVER_TRACE", None)
        else:
            os.environ["BASS_NEVER_TRACE"] = prev
    outT = np.concatenate([res.results[i]["out"].astype(np.float32)
                           for i in range(N_CORES)], axis=1)  # [D_OUT, B]
    return np.ascontiguousarray(outT.T)

